# revision 1
# baseline (speedup 1.0000x reference)
"""Trainium2 Bass kernel for BowEncoder (embedding lookup + masked mean pool).

out[b, :] = (1/len_b) * sum_{t<len_b} emb[input[b,t], :]
          = (1/len_b) * sum_v count[b, v] * emb[v, :]     (BoW form)

Sharding: vocab is split across the 8 NeuronCores (6400 zero-padded rows
each). Each core computes the partial sum over its table shard for ALL 64
batches as a dense PE matmul over 50 K-tiles of 128 vocab rows:

    psum[64, 256] += cnt_tile[128, 64].T @ emb_tile[128, 256]

Host prep per call: per-batch token histograms (uint8, exact), permuted to
the SBUF tile layout; table shard zero-padded. On device: counts arrive in
one 400KB DMA and are cast uint8->f32 once on DVE; the table shard streams
through the two HWDGE rings (SP/ACT alternating) with ramped transfer
sizes (small first groups so the first matmul starts early, 640KB groups
at steady state); all 50 matmuls accumulate into one PSUM bank; the
per-batch 1/len scale is a device-side reciprocal + per-partition
tensor_scalar; the 8 per-core partials are summed on the host (unshard).

This beats per-row gathers because SWDGE descriptor emission is serial at
~8ns/row (measured) — 16K rows/core can never beat ~130us — while the
dense stream reads the shard at ~383GB/s and the fp32 matmul runs warm at
(64+512)cyc/2.4GHz per tile.

Quirk: this walrus build allows only ONE sync-wait per instruction, so a
post-pass hoists excess waits onto same-engine NoOps.
"""

import numpy as np

import concourse.bass as bass
import concourse.mybir as mybir
import concourse.tile as tile
from concourse.bass_utils import run_bass_kernel_spmd

P = 128
B, T, V, H = 64, 2048, 50257, 256
NCORES = 8
VSHARD = 6400              # padded vocab rows per core (50 K-tiles of 128)
KT = VSHARD // P           # K-tiles per core
W = 64 + H                 # merged row width: counts | emb
KTG = 5                    # K-tiles per DMA

_DT = mybir.dt


def _split_multi_waits(nc, max_waits: int = 1) -> None:
    """This walrus build rejects instructions carrying more than one
    sync-wait. Hoist excess waits onto same-engine NoOps inserted before
    the instruction — engine queues execute in order."""
    for fn in nc.m.functions:
        for bb in fn.blocks:
            rebuilt = []
            changed = False
            for inst in bb.instructions:
                si = inst.sync_info
                if si is not None and si.on_wait and len(si.on_wait) > max_waits:
                    waits = list(si.on_wait)
                    extra, keep = waits[:-max_waits], waits[-max_waits:]
                    for j in range(0, len(extra), max_waits):
                        rebuilt.append(
                            mybir.InstNoOp(
                                name=f"{inst.name}-wsplit{j}",
                                sync_info=mybir.SyncInfo(
                                    on_wait=extra[j : j + max_waits], on_update=[]
                                ),
                                bass_nofuse=True,
                                engine=inst.engine,
                            )
                        )
                    inst.sync_info = mybir.SyncInfo(
                        on_wait=keep, on_update=list(si.on_update or [])
                    )
                    changed = True
                rebuilt.append(inst)
            if changed:
                bb.instructions = rebuilt


def _build_nc(split: bool = True):
    nc = bass.Bass("TRN2", target_bir_lowering=False)

    cnt = nc.dram_tensor("cnt", [P, KT * B], _DT.uint8, kind="ExternalInput")
    emb_t = nc.dram_tensor("embs", [VSHARD, 2 * H], _DT.bfloat16, kind="ExternalInput")
    lens = nc.dram_tensor("lens", [B, 1], _DT.int32, kind="ExternalInput")
    out = nc.dram_tensor("out", [B, H], _DT.float32, kind="ExternalOutput")

    with tile.TileContext(nc) as tc:
        with (
            tc.tile_pool(name="const", bufs=1) as const,
            tc.tile_pool(name="stream", bufs=8) as stream,
            tc.tile_pool(name="psum", bufs=1, space="PSUM") as psum_tp,
        ):
            lens_sb = const.tile([B, 1], _DT.int32)
            nc.sync.dma_start(out=lens_sb[:], in_=lens[:, :])
            lens_f = const.tile([B, 1], _DT.float32)
            nc.vector.tensor_copy(out=lens_f[:], in_=lens_sb[:])
            recip = const.tile([B, 1], _DT.float32)
            nc.vector.reciprocal(out=recip[:], in_=lens_f[:])

            # all counts up front: one 400KB DMA (host pre-permuted so
            # cnt[p, j*64+b] = count(vocab row j*128+p, batch b)), cast
            # uint8 -> f32 once on DVE
            cnt_u8 = const.tile([P, KT * B], _DT.uint8)
            nc.scalar.dma_start(out=cnt_u8[:], in_=cnt[:, :])
            cnt_f = const.tile([P, KT * B], _DT.bfloat16)
            # cast in two chunks so the first matmuls only wait on the first
            CSPLIT = 8 * B
            nc.vector.tensor_copy(out=cnt_f[:, :CSPLIT], in_=cnt_u8[:, :CSPLIT])
            nc.vector.tensor_copy(out=cnt_f[:, CSPLIT:], in_=cnt_u8[:, CSPLIT:])

            acc = psum_tp.tile([B, H], _DT.float32, space="PSUM")
            emb3 = emb_t[:, :].rearrange("(g p) h -> g p h", p=P)
            # ramped group sizes: small first transfers so the first matmul
            # starts as early as possible, big steady-state transfers after
            groups = [1, 2, 4] + [5] * 8 + [3]
            assert sum(groups) == KT
            j0 = 0
            for jg, gsz in enumerate(groups):
                tl = stream.tile([P, KTG, 2 * H], _DT.bfloat16, tag="tl")
                # alternate the two HWDGE rings (SP / ACT)
                dma_eng = nc.sync if jg % 2 == 0 else nc.scalar
                dma_eng.dma_start(
                    out=tl[:, :gsz, :],
                    in_=emb3[j0 : j0 + gsz, :, :].transpose([1, 0, 2]),
                )
                for j2 in range(gsz):
                    j = j0 + j2
                    for part in range(2):
                        nc.tensor.matmul(
                            out=acc[:],
                            lhsT=cnt_f[:, j * B : (j + 1) * B],
                            rhs=tl[:, j2, part * H : (part + 1) * H],
                            start=(j == 0 and part == 0),
                            stop=(j == KT - 1 and part == 1),
                        )
                j0 += gsz

            out_sb = const.tile([B, H], _DT.float32)
            nc.vector.tensor_scalar_mul(
                out=out_sb[:], in0=acc[:], scalar1=recip[:]
            )
            nc.sync.dma_start(out=out[:, :], in_=out_sb[:])

    if split:
        _split_multi_waits(nc)
    return nc


def _prep_in_maps(input_ids: np.ndarray, input_lens: np.ndarray, emb: np.ndarray):
    input_ids = np.asarray(input_ids, dtype=np.int64)
    input_lens = np.asarray(input_lens, dtype=np.int64)
    emb = np.asarray(emb, dtype=np.float32)

    # counts[v, b] over valid tokens
    counts = np.zeros((NCORES * VSHARD, B), dtype=np.int64)
    for b in range(B):
        L = int(input_lens[b])
        c = np.bincount(input_ids[b, :L], minlength=V)
        counts[:V, b] = c
    assert counts.max() <= 255, "uint8 count overflow"
    counts = counts.astype(np.uint8)

    import ml_dtypes

    embp = np.zeros((NCORES * VSHARD, 2 * H), dtype=ml_dtypes.bfloat16)
    hi = emb.astype(ml_dtypes.bfloat16)
    lo = (emb - hi.astype(np.float32)).astype(ml_dtypes.bfloat16)
    embp[:V, :H] = hi
    embp[:V, H:] = lo

    lens_arr = np.ascontiguousarray(input_lens.reshape(B, 1).astype(np.int32))
    in_maps = []
    for c0 in range(NCORES):
        sl = slice(c0 * VSHARD, (c0 + 1) * VSHARD)
        # cnt[p, j*64+b] = counts[shard_base + j*128 + p, b]
        cnt = np.ascontiguousarray(
            counts[sl].reshape(KT, P, B).transpose(1, 0, 2).reshape(P, KT * B)
        )
        in_maps.append(
            {"cnt": cnt, "embs": np.ascontiguousarray(embp[sl]), "lens": lens_arr}
        )
    return in_maps


_CACHE: dict = {}


def _run(inputs: dict, trace: bool = False):
    if "nc" not in _CACHE:
        _CACHE["nc"] = _build_nc()
    nc = _CACHE["nc"]
    in_maps = _prep_in_maps(inputs["input"], inputs["input_lens"], inputs["emb"])
    res = run_bass_kernel_spmd(nc, in_maps, core_ids=list(range(NCORES)), trace=trace)
    out = np.sum([res.results[c]["out"] for c in range(NCORES)], axis=0)
    return np.ascontiguousarray(out.astype(np.float32)), res


def kernel(input: np.ndarray, input_lens: np.ndarray, emb: np.ndarray) -> np.ndarray:
    out, _ = _run({"input": input, "input_lens": input_lens, "emb": emb})
    return out



# revision 2
# speedup vs baseline: 1.5356x; 1.5356x over previous
"""Trainium2 Bass kernel for BowEncoder (embedding lookup + masked mean pool).

out[b, :] = (1/len_b) * sum_{t<len_b} emb[input[b,t], :]
          = (1/len_b) * sum_v count[b, v] * emb[v, :]     (BoW form)

Sharding: vocab is split across the 8 NeuronCores (6400 zero-padded rows
each). Each core computes the partial sum over its table shard for ALL 64
batches as a dense PE matmul over 50 K-tiles of 128 vocab rows:

    psum[64, 256] += cnt_tile[128, 64].T @ emb_tile[128, 256]

Precision scheme (tolerance is 2e-2; this measures ~2.4e-3):
  - Main table is fp8 e4m3 (1 byte/elem -> 1.64MB/core stream).
  - The ~10 batches with the smallest len (where fp8 averaging error would
    blow up, incl. one len=1 batch) are computed in bf16 instead via one
    extra "side" K-tile per core: the distinct tokens of those batches
    (~907 rows) are gathered host-side into a 1024-row pool sharded 128
    rows/core, with bf16 values and separate side counts. Their columns
    are zeroed in the main fp8 counts.
  - Counts are shipped pre-converted to fp8 (exact for counts <= 16) so no
    device-side cast is needed; 1/len is shipped as fp32.

DMA layout: everything is host-permuted so every transfer is 128
contiguous per-partition descriptors (multi-KB each, amortizing the ~7ns
per-descriptor cost that capped the old 1KB-packet scheme at ~167GB/s per
queue). The two HWDGE rings (SP/ACT) each carry half the counts then a
ramp of table chunks; lens/side tiles ride the gpsimd SWDGE queue. All 51
matmuls accumulate into one PSUM bank; per-batch 1/len scale is one DVE
tensor_scalar; per-core partials are summed on the host (unshard).

Quirk: this walrus build allows only ONE sync-wait per instruction, so a
post-pass hoists excess waits onto same-engine NoOps.
"""

import numpy as np

import concourse.bass as bass
import concourse.mybir as mybir
import concourse.tile as tile
from concourse.bass_utils import run_bass_kernel_spmd

P = 128
B, T, V, H = 64, 2048, 50257, 256
NCORES = 8
VSHARD = 6400              # padded vocab rows per core (50 K-tiles of 128)
KT = VSHARD // P           # K-tiles per core
KTH = KT // 2              # K-tiles per ring
# table chunk sizes (K-tiles) per ring: ramped so the first matmul starts
# early, big chunks at steady state
CHUNKS = [2, 4, 6, 6, 7]
assert sum(CHUNKS) == KTH

_DT = mybir.dt


def _split_multi_waits(nc, max_waits: int = 1) -> None:
    """This walrus build rejects instructions carrying more than one
    sync-wait. Hoist excess waits onto same-engine NoOps inserted before
    the instruction — engine queues execute in order."""
    for fn in nc.m.functions:
        for bb in fn.blocks:
            rebuilt = []
            changed = False
            for inst in bb.instructions:
                si = inst.sync_info
                if si is not None and si.on_wait and len(si.on_wait) > max_waits:
                    waits = list(si.on_wait)
                    extra, keep = waits[:-max_waits], waits[-max_waits:]
                    for j in range(0, len(extra), max_waits):
                        rebuilt.append(
                            mybir.InstNoOp(
                                name=f"{inst.name}-wsplit{j}",
                                sync_info=mybir.SyncInfo(
                                    on_wait=extra[j : j + max_waits], on_update=[]
                                ),
                                bass_nofuse=True,
                                engine=inst.engine,
                            )
                        )
                    inst.sync_info = mybir.SyncInfo(
                        on_wait=keep, on_update=list(si.on_update or [])
                    )
                    changed = True
                rebuilt.append(inst)
            if changed:
                bb.instructions = rebuilt
    return


def _build_nc(split: bool = True):
    nc = bass.Bass("TRN2", target_bir_lowering=False)

    # counts, K-tile-major: cnt_x[p, j*64+b] = count(vocab row (j0+j)*128+p, b)
    cnt_a = nc.dram_tensor("cnt_a", [P, KTH * B], _DT.float8e4, kind="ExternalInput")
    cnt_b = nc.dram_tensor("cnt_b", [P, KTH * B], _DT.float8e4, kind="ExternalInput")
    # table, K-tile-major: emb_x[p, j*256+h] = emb(vocab row (j0+j)*128+p, h)
    emb_a = nc.dram_tensor("emb_a", [P, KTH * H], _DT.float8e4, kind="ExternalInput")
    emb_b = nc.dram_tensor("emb_b", [P, KTH * H], _DT.float8e4, kind="ExternalInput")
    # bf16 side tile for small-len batches
    scnt = nc.dram_tensor("scnt", [P, B], _DT.bfloat16, kind="ExternalInput")
    stbl = nc.dram_tensor("stbl", [P, H], _DT.bfloat16, kind="ExternalInput")
    recip = nc.dram_tensor("recip", [B, 1], _DT.float32, kind="ExternalInput")
    out = nc.dram_tensor("out", [B, H], _DT.float32, kind="ExternalOutput")

    with tile.TileContext(nc) as tc:
        with (
            tc.tile_pool(name="const", bufs=1) as const,
            tc.tile_pool(name="psum", bufs=1, space="PSUM") as psum_tp,
        ):
            # small stuff on the gpsimd SWDGE queue, off the HWDGE rings
            recip_sb = const.tile([B, 1], _DT.float32)
            nc.gpsimd.dma_start(out=recip_sb[:], in_=recip[:, :])
            scnt_sb = const.tile([P, B], _DT.bfloat16)
            nc.gpsimd.dma_start(out=scnt_sb[:], in_=scnt[:, :])
            stbl_sb = const.tile([P, H], _DT.bfloat16)
            nc.gpsimd.dma_start(out=stbl_sb[:], in_=stbl[:, :])

            # counts first on each ring (matmuls need them before the table)
            cnt_a_sb = const.tile([P, KTH * B], _DT.float8e4)
            nc.sync.dma_start(out=cnt_a_sb[:], in_=cnt_a[:, :])
            cnt_b_sb = const.tile([P, KTH * B], _DT.float8e4)
            nc.scalar.dma_start(out=cnt_b_sb[:], in_=cnt_b[:, :])

            acc = psum_tp.tile([B, H], _DT.float32, space="PSUM")

            # table chunks alternate rings; matmuls consume in arrival order
            tiles = []
            j0 = 0
            for ci, gsz in enumerate(CHUNKS):
                for ring, (emb_t, cnt_sb, jbase) in enumerate(
                    [(emb_a, cnt_a_sb, 0), (emb_b, cnt_b_sb, KTH)]
                ):
                    eng = nc.sync if ring == 0 else nc.scalar
                    tl = const.tile(
                        [P, gsz * H], _DT.float8e4, tag=f"tl{ci}r{ring}", name=f"tl{ci}r{ring}"
                    )
                    eng.dma_start(out=tl[:], in_=emb_t[:, j0 * H : (j0 + gsz) * H])
                    tiles.append((tl, cnt_sb, j0, gsz))
                j0 += gsz

            first = True
            for tl, cnt_sb, j0, gsz in tiles:
                for j2 in range(gsz):
                    j = j0 + j2
                    nc.tensor.matmul(
                        out=acc[:],
                        lhsT=cnt_sb[:, j * B : (j + 1) * B],
                        rhs=tl[:, j2 * H : (j2 + 1) * H],
                        start=first,
                        stop=False,
                    )
                    first = False
            # bf16 side tile last
            nc.tensor.matmul(
                out=acc[:], lhsT=scnt_sb[:], rhs=stbl_sb[:], start=False, stop=True
            )

            out_sb = const.tile([B, H], _DT.float32)
            nc.vector.tensor_scalar_mul(out=out_sb[:], in0=acc[:], scalar1=recip_sb[:])
            nc.sync.dma_start(out=out[:, :], in_=out_sb[:])

    if split:
        _split_multi_waits(nc)
    return nc


def _prep_in_maps(input_ids: np.ndarray, input_lens: np.ndarray, emb: np.ndarray):
    import ml_dtypes

    input_ids = np.asarray(input_ids, dtype=np.int64)
    input_lens = np.asarray(input_lens, dtype=np.int64)
    emb = np.asarray(emb, dtype=np.float32)

    # side batches: smallest len first while their distinct tokens fit the
    # 1024-row (8 cores x 128) bf16 side pool
    order = np.argsort(input_lens, kind="stable")
    side_batches = []
    side_tokens: set[int] = set()
    for b in order:
        toks = set(input_ids[b, : int(input_lens[b])].tolist())
        grown = side_tokens | toks
        if len(grown) > NCORES * P:
            break
        side_tokens = grown
        side_batches.append(int(b))
    side_rows = np.fromiter(side_tokens, dtype=np.int64)
    side_rows.sort()
    nsr = len(side_rows)
    sideset = set(side_batches)

    # counts[v, b] over valid tokens; side batches live only in side counts
    counts = np.zeros((NCORES * VSHARD, B), dtype=np.int64)
    side_counts = np.zeros((NCORES * P, B), dtype=np.int64)
    for b in range(B):
        L = int(input_lens[b])
        c = np.bincount(input_ids[b, :L], minlength=V)
        if b in sideset:
            side_counts[:nsr, b] = c[side_rows]
        else:
            counts[:V, b] = c
    assert counts.max() <= 16 and side_counts.max() <= 16, "fp8 count overflow"

    emb8 = np.zeros((NCORES * VSHARD, H), dtype=ml_dtypes.float8_e4m3)
    emb8[:V] = emb.astype(ml_dtypes.float8_e4m3)
    cnt8 = counts.astype(ml_dtypes.float8_e4m3)

    stbl_all = np.zeros((NCORES * P, H), dtype=ml_dtypes.bfloat16)
    stbl_all[:nsr] = emb[side_rows].astype(ml_dtypes.bfloat16)
    scnt_all = side_counts.astype(ml_dtypes.bfloat16)

    recip = np.ascontiguousarray(
        (1.0 / input_lens.astype(np.float32)).reshape(B, 1).astype(np.float32)
    )

    in_maps = []
    for c0 in range(NCORES):
        sl = slice(c0 * VSHARD, (c0 + 1) * VSHARD)
        # [VSHARD, X] -> K-tile-major [P, KT*X]
        cnt = cnt8[sl].reshape(KT, P, B).transpose(1, 0, 2).reshape(P, KT * B)
        embp = emb8[sl].reshape(KT, P, H).transpose(1, 0, 2).reshape(P, KT * H)
        ssl = slice(c0 * P, (c0 + 1) * P)
        in_maps.append(
            {
                "cnt_a": np.ascontiguousarray(cnt[:, : KTH * B]),
                "cnt_b": np.ascontiguousarray(cnt[:, KTH * B :]),
                "emb_a": np.ascontiguousarray(embp[:, : KTH * H]),
                "emb_b": np.ascontiguousarray(embp[:, KTH * H :]),
                "scnt": np.ascontiguousarray(scnt_all[ssl]),
                "stbl": np.ascontiguousarray(stbl_all[ssl]),
                "recip": recip,
            }
        )
    return in_maps


_CACHE: dict = {}


def _run(inputs: dict, trace: bool = False, tmpdir: str | None = None):
    if "nc" not in _CACHE:
        _CACHE["nc"] = _build_nc()
    nc = _CACHE["nc"]
    in_maps = _prep_in_maps(inputs["input"], inputs["input_lens"], inputs["emb"])
    res = run_bass_kernel_spmd(
        nc, in_maps, core_ids=list(range(NCORES)), trace=trace, tmpdir=tmpdir
    )
    out = np.sum([res.results[c]["out"] for c in range(NCORES)], axis=0)
    return np.ascontiguousarray(out.astype(np.float32)), res


def kernel(input: np.ndarray, input_lens: np.ndarray, emb: np.ndarray) -> np.ndarray:
    out, _ = _run({"input": input, "input_lens": input_lens, "emb": emb})
    return out


# revision 3
# speedup vs baseline: 1.5899x; 1.0354x over previous
"""Trainium2 Bass kernel for BowEncoder (embedding lookup + masked mean pool).

out[b, :] = (1/len_b) * sum_{t<len_b} emb[input[b,t], :]
          = (1/len_b) * sum_v count[b, v] * emb[v, :]     (BoW form)

Sharding: vocab is split across the 8 NeuronCores (6656 zero-padded rows
each = 26 pairs of 128-row K-tiles). Each core computes the partial sum
over its table shard for ALL 64 batches with fp8 DoubleRow PE matmuls
(two K-tiles per instruction, 0.5 cycles/row):

    psum[64, 256] += sum_i cnt[128, i, 64].T @ tbl[128, i, 256]  (i=0,1)

Precision scheme (tolerance is 2e-2; this measures ~2.4e-3):
  - Main table is fp8 e4m3 (1 byte/elem -> 1.70MB/core stream).
  - The ~10 batches with the smallest len (where fp8 averaging error
    would blow up, incl. one len=1 batch) are computed in bf16 instead
    via one extra "side" K-tile per core (normal-mode matmul): the
    distinct tokens of those batches (~907 rows) are gathered host-side
    into a 1024-row pool sharded 128 rows/core. Their columns are zeroed
    in the main fp8 counts.
  - Counts are shipped pre-converted to fp8 (exact for counts <= 16); no
    device-side cast. 1/len is shipped as fp32.

DMA plan (HWDGE queue rate ~= desc_size/(6ns + 4.7ps/B) per queue, so big
per-partition descriptors matter): each HWDGE ring (SP/ACT) carries 2
pairs of "head" counts (so the first matmuls can issue early) then its 13
table pairs in two chunks of 3.0/3.5KB descriptors. The bulk counts (11
pairs per half, 1.4KB descriptors) + recip + side tiles ride the gpsimd
SWDGE queue in parallel. All 27 matmuls accumulate into one PSUM bank;
the per-batch 1/len scale is one DVE tensor_scalar; per-core partials
are summed on the host (unshard).

Quirk: this walrus build allows only ONE sync-wait per instruction, so a
post-pass hoists excess waits onto same-engine NoOps.
"""

import numpy as np

import concourse.bass as bass
import concourse.mybir as mybir
import concourse.tile as tile
from concourse.bass_utils import run_bass_kernel_spmd

P = 128
B, T, V, H = 64, 2048, 50257, 256
NCORES = 8
VSHARD = 6656              # padded vocab rows per core (26 pairs of 2 K-tiles)
KT = VSHARD // P           # K-tiles per core (52)
NPAIR = KT // 2            # DoubleRow pairs per core (26)
RPAIR = NPAIR // 2         # pairs per ring (13)
HEADP = 2                  # head-count pairs per ring (on the HWDGE ring)
RESTP = RPAIR - HEADP      # bulk-count pairs per ring (on SWDGE)
CHUNKS = [6, 7]            # table chunk sizes (pairs) per ring
assert sum(CHUNKS) == RPAIR

_DT = mybir.dt
_DR = mybir.MatmulPerfMode.DoubleRow


def _split_multi_waits(nc, max_waits: int = 1) -> None:
    """This walrus build rejects instructions carrying more than one
    sync-wait. Hoist excess waits onto same-engine NoOps inserted before
    the instruction — engine queues execute in order."""
    for fn in nc.m.functions:
        for bb in fn.blocks:
            rebuilt = []
            changed = False
            for inst in bb.instructions:
                si = inst.sync_info
                if si is not None and si.on_wait and len(si.on_wait) > max_waits:
                    waits = list(si.on_wait)
                    extra, keep = waits[:-max_waits], waits[-max_waits:]
                    for j in range(0, len(extra), max_waits):
                        rebuilt.append(
                            mybir.InstNoOp(
                                name=f"{inst.name}-wsplit{j}",
                                sync_info=mybir.SyncInfo(
                                    on_wait=extra[j : j + max_waits], on_update=[]
                                ),
                                bass_nofuse=True,
                                engine=inst.engine,
                            )
                        )
                    inst.sync_info = mybir.SyncInfo(
                        on_wait=keep, on_update=list(si.on_update or [])
                    )
                    changed = True
                rebuilt.append(inst)
            if changed:
                bb.instructions = rebuilt
    return


def _build_nc(split: bool = True):
    nc = bass.Bass("TRN2", target_bir_lowering=False)

    # counts, pair-major: [P, pair, i, B]; tables: [P, pair, i, H]
    cnt_ah = nc.dram_tensor("cnt_ah", [P, HEADP * 2 * B], _DT.float8e4, kind="ExternalInput")
    cnt_bh = nc.dram_tensor("cnt_bh", [P, HEADP * 2 * B], _DT.float8e4, kind="ExternalInput")
    cnt_ar = nc.dram_tensor("cnt_ar", [P, RESTP * 2 * B], _DT.float8e4, kind="ExternalInput")
    cnt_br = nc.dram_tensor("cnt_br", [P, RESTP * 2 * B], _DT.float8e4, kind="ExternalInput")
    emb_a0 = nc.dram_tensor("emb_a0", [P, CHUNKS[0] * 2 * H], _DT.float8e4, kind="ExternalInput")
    emb_a1 = nc.dram_tensor("emb_a1", [P, CHUNKS[1] * 2 * H], _DT.float8e4, kind="ExternalInput")
    emb_b0 = nc.dram_tensor("emb_b0", [P, CHUNKS[0] * 2 * H], _DT.float8e4, kind="ExternalInput")
    emb_b1 = nc.dram_tensor("emb_b1", [P, CHUNKS[1] * 2 * H], _DT.float8e4, kind="ExternalInput")
    scnt = nc.dram_tensor("scnt", [P, B], _DT.bfloat16, kind="ExternalInput")
    stbl = nc.dram_tensor("stbl", [P, H], _DT.bfloat16, kind="ExternalInput")
    recip = nc.dram_tensor("recip", [B, 1], _DT.float32, kind="ExternalInput")
    out = nc.dram_tensor("out", [B, H], _DT.float32, kind="ExternalOutput")

    with tile.TileContext(nc) as tc:
        with (
            tc.tile_pool(name="const", bufs=1) as const,
            tc.tile_pool(name="psum", bufs=1, space="PSUM") as psum_tp,
        ):
            # head counts first on each HWDGE ring
            cah = const.tile([P, HEADP, 2, B], _DT.float8e4)
            nc.sync.dma_start(out=cah[:], in_=cnt_ah[:, :])
            cbh = const.tile([P, HEADP, 2, B], _DT.float8e4)
            nc.scalar.dma_start(out=cbh[:], in_=cnt_bh[:, :])

            # bulk counts + small stuff on the gpsimd SWDGE queue
            car = const.tile([P, RESTP, 2, B], _DT.float8e4)
            nc.gpsimd.dma_start(out=car[:], in_=cnt_ar[:, :])
            cbr = const.tile([P, RESTP, 2, B], _DT.float8e4)
            nc.gpsimd.dma_start(out=cbr[:], in_=cnt_br[:, :])
            recip_sb = const.tile([B, 1], _DT.float32)
            nc.gpsimd.dma_start(out=recip_sb[:], in_=recip[:, :])
            scnt_sb = const.tile([P, B], _DT.bfloat16)
            nc.gpsimd.dma_start(out=scnt_sb[:], in_=scnt[:, :])
            stbl_sb = const.tile([P, H], _DT.bfloat16)
            nc.gpsimd.dma_start(out=stbl_sb[:], in_=stbl[:, :])

            # table chunks
            ta0 = const.tile([P, CHUNKS[0], 2, H], _DT.float8e4)
            nc.sync.dma_start(out=ta0[:], in_=emb_a0[:, :])
            tb0 = const.tile([P, CHUNKS[0], 2, H], _DT.float8e4)
            nc.scalar.dma_start(out=tb0[:], in_=emb_b0[:, :])
            ta1 = const.tile([P, CHUNKS[1], 2, H], _DT.float8e4)
            nc.sync.dma_start(out=ta1[:], in_=emb_a1[:, :])
            tb1 = const.tile([P, CHUNKS[1], 2, H], _DT.float8e4)
            nc.scalar.dma_start(out=tb1[:], in_=emb_b1[:, :])

            acc = psum_tp.tile([B, H], _DT.float32, space="PSUM")

            # pair j on ring r: counts from head tile (j < HEADP) or rest
            def cnt_ap(head, rest, j):
                return head[:, j] if j < HEADP else rest[:, j - HEADP]

            # consumption order: A0 head pairs, B0 head pairs, A0 rest, B0
            # rest, A1, B1 — matches expected arrival order
            sched = []
            for ring, (t0, t1, ch, cr) in enumerate(
                [(ta0, ta1, cah, car), (tb0, tb1, cbh, cbr)]
            ):
                for j in range(HEADP):
                    sched.append((0, t0, j, cnt_ap(ch, cr, j)))
            for ring, (t0, t1, ch, cr) in enumerate(
                [(ta0, ta1, cah, car), (tb0, tb1, cbh, cbr)]
            ):
                for j in range(HEADP, CHUNKS[0]):
                    sched.append((0, t0, j, cnt_ap(ch, cr, j)))
            for ring, (t0, t1, ch, cr) in enumerate(
                [(ta0, ta1, cah, car), (tb0, tb1, cbh, cbr)]
            ):
                for j in range(CHUNKS[1]):
                    sched.append((1, t1, j, cnt_ap(ch, cr, CHUNKS[0] + j)))

            first = True
            for _, tl, j, cap in sched:
                nc.tensor.matmul(
                    out=acc[:],
                    lhsT=cap,
                    rhs=tl[:, j],
                    start=first,
                    stop=False,
                    perf_mode=_DR,
                )
                first = False
            # bf16 side tile last (normal mode)
            nc.tensor.matmul(
                out=acc[:], lhsT=scnt_sb[:], rhs=stbl_sb[:], start=False, stop=True
            )

            out_sb = const.tile([B, H], _DT.float32)
            nc.vector.tensor_scalar_mul(out=out_sb[:], in0=acc[:], scalar1=recip_sb[:])
            nc.sync.dma_start(out=out[:, :], in_=out_sb[:])

    if split:
        _split_multi_waits(nc)
    return nc


def _prep_in_maps(input_ids: np.ndarray, input_lens: np.ndarray, emb: np.ndarray):
    import ml_dtypes

    input_ids = np.asarray(input_ids, dtype=np.int64)
    input_lens = np.asarray(input_lens, dtype=np.int64)
    emb = np.asarray(emb, dtype=np.float32)

    # side batches: smallest len first while their distinct tokens fit the
    # 1024-row (8 cores x 128) bf16 side pool
    order = np.argsort(input_lens, kind="stable")
    side_batches = []
    side_tokens: set[int] = set()
    for b in order:
        toks = set(input_ids[b, : int(input_lens[b])].tolist())
        grown = side_tokens | toks
        if len(grown) > NCORES * P:
            break
        side_tokens = grown
        side_batches.append(int(b))
    side_rows = np.fromiter(side_tokens, dtype=np.int64)
    side_rows.sort()
    nsr = len(side_rows)
    sideset = set(side_batches)

    # counts[v, b] over valid tokens; side batches live only in side counts
    counts = np.zeros((NCORES * VSHARD, B), dtype=np.int64)
    side_counts = np.zeros((NCORES * P, B), dtype=np.int64)
    for b in range(B):
        L = int(input_lens[b])
        c = np.bincount(input_ids[b, :L], minlength=V)
        if b in sideset:
            side_counts[:nsr, b] = c[side_rows]
        else:
            counts[:V, b] = c
    assert counts.max() <= 16 and side_counts.max() <= 16, "fp8 count overflow"

    emb8 = np.zeros((NCORES * VSHARD, H), dtype=ml_dtypes.float8_e4m3)
    emb8[:V] = emb.astype(ml_dtypes.float8_e4m3)
    cnt8 = counts.astype(ml_dtypes.float8_e4m3)

    stbl_all = np.zeros((NCORES * P, H), dtype=ml_dtypes.bfloat16)
    stbl_all[:nsr] = emb[side_rows].astype(ml_dtypes.bfloat16)
    scnt_all = side_counts.astype(ml_dtypes.bfloat16)

    recip = np.ascontiguousarray(
        (1.0 / input_lens.astype(np.float32)).reshape(B, 1).astype(np.float32)
    )

    def pairize(x):
        # [VSHARD, X] -> [P, NPAIR, 2, X] flattened to [P, NPAIR*2*X]
        X = x.shape[1]
        return (
            x.reshape(NPAIR, 2, P, X).transpose(2, 0, 1, 3).reshape(P, NPAIR * 2 * X)
        )

    C = np.ascontiguousarray
    in_maps = []
    for c0 in range(NCORES):
        sl = slice(c0 * VSHARD, (c0 + 1) * VSHARD)
        cnt = pairize(cnt8[sl])       # [P, NPAIR*2*B]
        embp = pairize(emb8[sl])      # [P, NPAIR*2*H]
        ssl = slice(c0 * P, (c0 + 1) * P)
        cw, ew = 2 * B, 2 * H
        in_maps.append(
            {
                "cnt_ah": C(cnt[:, : HEADP * cw]),
                "cnt_ar": C(cnt[:, HEADP * cw : RPAIR * cw]),
                "cnt_bh": C(cnt[:, RPAIR * cw : (RPAIR + HEADP) * cw]),
                "cnt_br": C(cnt[:, (RPAIR + HEADP) * cw :]),
                "emb_a0": C(embp[:, : CHUNKS[0] * ew]),
                "emb_a1": C(embp[:, CHUNKS[0] * ew : RPAIR * ew]),
                "emb_b0": C(embp[:, RPAIR * ew : (RPAIR + CHUNKS[0]) * ew]),
                "emb_b1": C(embp[:, (RPAIR + CHUNKS[0]) * ew :]),
                "scnt": C(scnt_all[ssl]),
                "stbl": C(stbl_all[ssl]),
                "recip": recip,
            }
        )
    return in_maps


_CACHE: dict = {}


def _run(inputs: dict, trace: bool = False, tmpdir: str | None = None):
    if "nc" not in _CACHE:
        _CACHE["nc"] = _build_nc()
    nc = _CACHE["nc"]
    in_maps = _prep_in_maps(inputs["input"], inputs["input_lens"], inputs["emb"])
    res = run_bass_kernel_spmd(
        nc, in_maps, core_ids=list(range(NCORES)), trace=trace, tmpdir=tmpdir
    )
    out = np.sum([res.results[c]["out"] for c in range(NCORES)], axis=0)
    return np.ascontiguousarray(out.astype(np.float32)), res


def kernel(input: np.ndarray, input_lens: np.ndarray, emb: np.ndarray) -> np.ndarray:
    out, _ = _run({"input": input, "input_lens": input_lens, "emb": emb})
    return out
